# revision 11
# baseline (speedup 1.0000x reference)
"""CostVolume kernel for Trainium2 (8 NeuronCores, Bass/Tile).

Math: the reference computes a 9x9-displacement correlation cost volume and
scatters it into out[b, r', c', r, c].  Substituting r' = r + di - 4,
c' = c + dj - 4 shows the output is just a banded Gram matrix:

    out[b, r', c', r, c] = (sum_ch feat2[b,ch,r',c'] * feat1[b,ch,r,c])
                           * 1[|r'-r| <= 4] * 1[|c'-c| <= 4]

Only ~2% of the (B,H,W,H,W) output is inside the band; the rest is zeros.
The device computes and writes ONLY the band (as bf16) and the host
scatters it into the dense float32 result during the unshard step.  This
cuts per-core HBM traffic from ~38 MB (dense fp32 output) to ~4 MB.

Sharding: 8 cores = 4 batches x 2 column-halves (c' in [0,32) / [32,64)).
A single SPMD program serves all 8 cores; per-core differences (feat2
slice/order, feat1 column window + zero padding) are baked into the
input arrays host-side.

Per core: 16 "quads" of 8 consecutive r' x 16 consecutive c' = 128 PSUM
partitions (p = rg*16 + cj).  Quad (kr, kc) computes
    psum[128, 16*24] = f2_quad[256,128]^T @ f1_win[256, 16, 24]
where the f1 window is padded rows [8kr, 8kr+16) x padded cols
[16kc, 16kc+24) of the core's (72 x 40) zero-padded feat1 slab — a
strided 2D slice.  The 8x16 quad shape minimizes the window area
((RQ+8)*(CQ+8)) and hence PE cycles and output bytes.  2 bf16 matmuls
per quad (one per 128-channel half); single-term bf16 gives ~3e-3 rel
error vs the 2e-2 gate.  Every partition's valid 9x9 neighborhood lives
at blk = rg+di, wc = cj+dj — the host gathers it with one as_strided.

Schedule (the NEFF epilogue serially resets ~50 semaphores at ~115 ns
each — a fixed ~6.5 us tail — so the work window is what we optimize):
  - inputs stream in 4 chunks (f2 on the SP queue, f1 on the Activation
    queue) sized so quad 0 starts ASAP and the PE never starves.
  - 8 warmup matmuls bridge the PE from the preamble barrier to the
    first real matmul with no idle gap: the HAM clock gate reaches
    2.4 GHz only after ~5.5 us of near-uninterrupted PE activity.
  - one Vector/Scalar copy per quad (alternating) casts psum -> bf16.
  - output: 8 pair-level DMAs on the SP queue (full windows; the host
    ignores out-of-band lanes).
"""

import numpy as np

B, C, H, W = 4, 256, 64, 64
MD = 4
N_CORES = 8
CSH = W // 2          # 32 c' columns per core
WC = CSH + 2 * MD     # 40-wide padded c window per core
RQ = 8                # r' rows per quad
CQ = 16               # c' cols per quad
NQ = 16               # quads per core: (64/RQ) * (32/CQ)
RB = RQ + 2 * MD      # 16 r-blocks in a quad's window
CB = CQ + 2 * MD      # 24 c-cols in a quad's window
NWIN = RB * CB        # 384 psum columns per quad
HP = H + 2 * MD       # 72 padded f1 rows
NWARM = 13            # PE warmup matmuls
NPAIR = NQ // 2       # 8 quad pairs (pair = one kr slab)

_COMPILED = None      # compiled Bacc program cache across kernel() calls


def _build_program():
    import concourse.bacc as bacc
    import concourse.tile as tile
    from concourse import mybir

    f32 = mybir.dt.float32
    bf16 = mybir.dt.bfloat16

    nc = bacc.Bacc("TRN2", target_bir_lowering=False, debug=False,
                   num_devices=N_CORES)

    f2d = nc.dram_tensor("f2", [C, H * CSH], bf16, kind="ExternalInput").ap()
    f1d = nc.dram_tensor("f1", [C, H * WC], bf16, kind="ExternalInput").ap()
    outd = nc.dram_tensor("out", [NQ, 128, NWIN], bf16,
                          kind="ExternalOutput").ap()

    # input chunks (quads ordered k = 2*kr + kc):
    #   f1 padded-row chunks [0,16), [16,32), [32,56), [56,72)
    #   f2 quad chunks       [0,2),  [2,6),   [6,12),  [12,16)
    f1_cuts = [0, 16 * WC, 32 * WC, 56 * WC, HP * WC]  # sbuf padded rows
    f1_src_cuts = [max(0, c - MD * WC) for c in f1_cuts[:-1]] + [H * WC]
    f2_cuts = [0, 2 * 128, 6 * 128, 12 * 128, 16 * 128]

    with tile.TileContext(nc) as tc:
        with (
            tc.tile_pool(name="persist", bufs=1) as persist,
            tc.tile_pool(name="psum", bufs=8, space="PSUM") as psum_pool,
        ):
            # PE warmup (see module docstring); operand contents irrelevant.
            warm_t = persist.tile([128, NWIN], bf16, tag="warm")
            nc.gpsimd.memset(warm_t[:], 0.0)
            for _ in range(NWARM):
                wp = psum_pool.tile([128, NWIN], f32, tag="ps", name="wp")
                nc.tensor.matmul(wp[:], warm_t[:, 0:128], warm_t[:],
                                 start=True, stop=True)

            f2_t = persist.tile([128, 2 * H * CSH], bf16, tag="f2")
            f1_t = persist.tile([128, 2 * HP * WC], bf16, tag="f1")
            f2_src = f2d.rearrange("(h p) n -> p h n", h=2)
            f2_dst = f2_t[:, :].rearrange("p (h n) -> p h n", h=2)
            f1_src = f1d.rearrange("(h p) n -> p h n", h=2)
            f1_dst = f1_t[:, :].rearrange("p (h n) -> p h n", h=2)
            # zero the 4 pad rows at top/bottom of each f1 half (rows are
            # only touched by quads kr=0 / kr=7)
            nc.gpsimd.memset(f1_t[:, 0:MD * WC], 0.0)
            nc.gpsimd.memset(f1_t[:, (MD + H) * WC:HP * WC], 0.0)
            nc.gpsimd.memset(f1_t[:, HP * WC:HP * WC + MD * WC], 0.0)
            nc.gpsimd.memset(f1_t[:, (HP + MD + H) * WC:], 0.0)
            for c in range(4):
                s2 = slice(f2_cuts[c], f2_cuts[c + 1])
                s1s = slice(f1_src_cuts[c], f1_src_cuts[c + 1])
                s1d = slice(f1_src_cuts[c] + MD * WC,
                            f1_src_cuts[c + 1] + MD * WC)
                nc.sync.dma_start(out=f2_dst[:, :, s2], in_=f2_src[:, :, s2])
                nc.scalar.dma_start(out=f1_dst[:, :, s1d], in_=f1_src[:, :, s1s])

            band = [persist.tile([128, 2 * NWIN], bf16, tag=f"band{p}",
                                 name=f"band{p}")
                    for p in range(NPAIR)]

            for k in range(NQ):
                kr, kc = divmod(k, 2)
                ps = psum_pool.tile([128, NWIN], f32, tag="ps")
                for h in range(2):
                    win = (f1_t[:, h * HP * WC:(h + 1) * HP * WC]
                           .rearrange("p (r c) -> p r c", c=WC)
                           [:, RQ * kr:RQ * kr + RB, CQ * kc:CQ * kc + CB])
                    nc.tensor.matmul(
                        ps[:],
                        f2_t[:, h * H * CSH + 128 * k:
                             h * H * CSH + 128 * (k + 1)],
                        win,
                        start=(h == 0), stop=(h == 1),
                    )
                dst = band[kr][:, kc * NWIN:(kc + 1) * NWIN]
                if k % 2:
                    nc.scalar.copy(out=dst, in_=ps[:])
                else:
                    nc.vector.tensor_copy(out=dst, in_=ps[:])

                if kc == 1:
                    nc.sync.dma_start(
                        out=outd[2 * kr:2 * (kr + 1)].transpose([1, 0, 2]),
                        in_=band[kr][:, :])
                if k in (1, 5):
                    for _ in range(2):
                        wp = psum_pool.tile([128, NWIN], f32, tag="ps",
                                            name="wp2")
                        nc.tensor.matmul(wp[:], warm_t[:, 0:128], warm_t[:],
                                         start=True, stop=True)

    nc.compile()
    return nc


def _shard_inputs(feat1, feat2):
    """Per-core input dicts. Core i = (batch i//2, column-half i%2)."""
    import ml_dtypes
    bf = ml_dtypes.bfloat16
    in_maps = []
    for i in range(N_CORES):
        b, ch = divmod(i, 2)
        clo = ch * CSH
        # f2 in quad-partition order: (ch, kr, kc, rg, cj)
        f2s = (feat2[b, :, :, clo:clo + CSH]
               .reshape(C, H // RQ, RQ, CSH // CQ, CQ)
               .transpose(0, 1, 3, 2, 4)
               .reshape(C, H * CSH)).astype(bf)
        f2s = np.ascontiguousarray(f2s)
        # f1: columns [clo-4, clo+36) with zeros outside the image; the
        # 4 pad rows top/bottom are memset on-device
        f1p = np.zeros((C, H, WC), np.float32)
        lo, hi = max(0, clo - MD), min(W, clo + CSH + MD)
        f1p[:, :, lo - (clo - MD):hi - (clo - MD)] = feat1[b, :, :, lo:hi]
        in_maps.append({"f2": f2s, "f1": f1p.reshape(C, H * WC).astype(bf)})
    return in_maps


def _unshard(results):
    """Scatter the per-core bf16 bands into the dense (B,H,W,H,W) output."""
    P = 2 * MD + 1
    V = np.empty((B, H, W, P, P), np.float32)
    for i in range(N_CORES):
        b, ch = divmod(i, 2)
        a = np.asarray(results[i]["out"]).astype(np.float32)
        a = a.reshape(H // RQ, CSH // CQ, RQ, CQ, RB, CB)
        s = a.strides
        # Vc[kr, kc, rg, cj, di, dj] = a[kr, kc, rg, cj, rg + di, cj + dj]
        Vc = np.lib.stride_tricks.as_strided(
            a, shape=(H // RQ, CSH // CQ, RQ, CQ, P, P),
            strides=(s[0], s[1], s[2] + s[4], s[3] + s[5], s[4], s[5]))
        V[b, :, ch * CSH:(ch + 1) * CSH] = (
            Vc.transpose(0, 2, 1, 3, 4, 5).reshape(H, CSH, P, P))

    out5 = np.zeros((B, H, W, H, W), np.float32)
    so = out5.strides
    for di in range(P):
        ddi = di - MD
        rlo, rhi = max(0, -ddi), min(H, H - ddi)
        for dj in range(P):
            ddj = dj - MD
            clo2, chi2 = max(0, -ddj), min(W, W - ddj)
            src = V[:, rlo:rhi, clo2:chi2, di, dj]
            base = out5[:, rlo:, clo2:, rlo + ddi:, clo2 + ddj:]
            tgt = np.lib.stride_tricks.as_strided(
                base, shape=(B, rhi - rlo, chi2 - clo2),
                strides=(so[0], so[1] + so[3], so[2] + so[4]))
            tgt[...] = src
    return out5.reshape(B, H * W, H, W)


def run(feat1, feat2, trace=False, trace_cores=None):
    """Returns (full output (B, H*W, H, W) float32, exec_time_ns or None)."""
    global _COMPILED
    from concourse.bass_utils import run_bass_kernel_spmd

    feat1 = np.asarray(feat1, dtype=np.float32)
    feat2 = np.asarray(feat2, dtype=np.float32)
    assert feat1.shape == (B, C, H, W) and feat2.shape == (B, C, H, W)

    if _COMPILED is None:
        _COMPILED = _build_program()
    nc = _COMPILED

    in_maps = _shard_inputs(feat1, feat2)
    res = run_bass_kernel_spmd(
        nc, in_maps, core_ids=list(range(N_CORES)),
        trace=trace, trace_cores=trace_cores,
    )
    return _unshard(res.results), res.exec_time_ns


def kernel(feat1, feat2):
    out, _ = run(feat1, feat2, trace=False)
    return out


# revision 12
# speedup vs baseline: 1.0897x; 1.0897x over previous
"""CostVolume kernel for Trainium2 (8 NeuronCores, Bass/Tile).

Math: the reference computes a 9x9-displacement correlation cost volume and
scatters it into out[b, r', c', r, c].  Substituting r' = r + di - 4,
c' = c + dj - 4 shows the output is just a banded Gram matrix:

    out[b, r', c', r, c] = (sum_ch feat2[b,ch,r',c'] * feat1[b,ch,r,c])
                           * 1[|r'-r| <= 4] * 1[|c'-c| <= 4]

Only ~2% of the (B,H,W,H,W) output is inside the band; the rest is zeros.
The device computes and writes ONLY the band (as bf16) and the host
scatters it into the dense float32 result during the unshard step.  This
cuts per-core HBM traffic from ~38 MB (dense fp32 output) to ~4 MB.

Sharding: 8 cores = 4 batches x 2 column-halves (c' in [0,32) / [32,64)).
A single SPMD program serves all 8 cores; per-core differences (feat2
slice/order, feat1 column window + zero padding) are baked into the
input arrays host-side.

Per core: 16 "quads" of 8 consecutive r' x 16 consecutive c' = 128 PSUM
partitions (p = rg*16 + cj).  Quad (kr, kc) computes
    psum[128, 16*24] = f2_quad[256,128]^T @ f1_win[256, 16, 24]
where the f1 window is padded rows [8kr, 8kr+16) x padded cols
[16kc, 16kc+24) of the core's (72 x 40) zero-padded feat1 slab — a
strided 2D slice.  The 8x16 quad shape minimizes the window area
((RQ+8)*(CQ+8)) and hence PE cycles and output bytes.  2 bf16 matmuls
per quad (one per 128-channel half); single-term bf16 gives ~3e-3 rel
error vs the 2e-2 gate.  Every partition's valid 9x9 neighborhood lives
at blk = rg+di, wc = cj+dj — the host gathers it with one as_strided.

Schedule (the NEFF epilogue serially resets ~50 semaphores at ~115 ns
each — a fixed ~6.5 us tail — so the work window is what we optimize):
  - inputs stream in 4 chunks (f2 on the SP queue, f1 on the Activation
    queue) sized so quad 0 starts ASAP and the PE never starves.
  - 8 warmup matmuls bridge the PE from the preamble barrier to the
    first real matmul with no idle gap: the HAM clock gate reaches
    2.4 GHz only after ~5.5 us of near-uninterrupted PE activity.
  - one Vector/Scalar copy per quad (alternating) casts psum -> bf16.
  - output: 8 pair-level DMAs on the SP queue (full windows; the host
    ignores out-of-band lanes).
"""

import numpy as np

B, C, H, W = 4, 256, 64, 64
MD = 4
N_CORES = 8
CSH = W // 2          # 32 c' columns per core
WC = CSH + 2 * MD     # 40-wide padded c window per core
RQ = 8                # r' rows per quad
CQ = 16               # c' cols per quad
NQ = 16               # quads per core: (64/RQ) * (32/CQ)
RB = RQ + 2 * MD      # 16 r-blocks in a quad's window
CB = CQ + 2 * MD      # 24 c-cols in a quad's window
NWIN = RB * CB        # 384 psum columns per quad
HP = H + 2 * MD       # 72 padded f1 rows
NWARM = 10            # PE warmup matmuls
NPAIR = NQ // 2       # 8 quad pairs (pair = one kr slab)

_COMPILED = None      # compiled Bacc program cache across kernel() calls


def _build_program():
    import concourse.bacc as bacc
    import concourse.tile as tile
    from concourse import mybir

    f32 = mybir.dt.float32
    bf16 = mybir.dt.bfloat16

    nc = bacc.Bacc("TRN2", target_bir_lowering=False, debug=False,
                   num_devices=N_CORES)

    f2d = nc.dram_tensor("f2", [C, H * CSH], bf16, kind="ExternalInput").ap()
    f1d = nc.dram_tensor("f1", [C, H * WC], bf16, kind="ExternalInput").ap()
    outd = nc.dram_tensor("out", [NQ, 128, NWIN], bf16,
                          kind="ExternalOutput").ap()

    # input chunks (quads ordered k = 2*kr + kc):
    #   f1 padded-row chunks [0,16), [16,32), [32,56), [56,72)
    #   f2 quad chunks       [0,2),  [2,6),   [6,12),  [12,16)
    f1_cuts = [0, 16 * WC, 32 * WC, 56 * WC, HP * WC]  # sbuf padded rows
    f1_src_cuts = [max(0, c - MD * WC) for c in f1_cuts[:-1]] + [H * WC]
    f2_cuts = [0, 2 * 128, 6 * 128, 12 * 128, 16 * 128]

    with tile.TileContext(nc) as tc:
        with (
            tc.tile_pool(name="persist", bufs=1) as persist,
            tc.tile_pool(name="psum", bufs=8, space="PSUM") as psum_pool,
        ):
            # PE warmup (see module docstring); operand contents irrelevant.
            warm_t = persist.tile([128, NWIN], bf16, tag="warm")
            nc.gpsimd.memset(warm_t[:], 0.0)
            for _ in range(NWARM):
                wp = psum_pool.tile([128, NWIN], f32, tag="ps", name="wp")
                nc.tensor.matmul(wp[:], warm_t[:, 0:128], warm_t[:],
                                 start=True, stop=True)

            f2_t = persist.tile([128, 2 * H * CSH], bf16, tag="f2")
            f1_t = persist.tile([128, 2 * HP * WC], bf16, tag="f1")
            f2_src = f2d.rearrange("(h p) n -> p h n", h=2)
            f2_dst = f2_t[:, :].rearrange("p (h n) -> p h n", h=2)
            f1_src = f1d.rearrange("(h p) n -> p h n", h=2)
            f1_dst = f1_t[:, :].rearrange("p (h n) -> p h n", h=2)
            # zero the 4 pad rows at top/bottom of each f1 half (rows are
            # only touched by quads kr=0 / kr=7)
            nc.gpsimd.memset(f1_t[:, 0:MD * WC], 0.0)
            nc.gpsimd.memset(f1_t[:, (MD + H) * WC:HP * WC], 0.0)
            nc.gpsimd.memset(f1_t[:, HP * WC:HP * WC + MD * WC], 0.0)
            nc.gpsimd.memset(f1_t[:, (HP + MD + H) * WC:], 0.0)
            for c in range(4):
                s2 = slice(f2_cuts[c], f2_cuts[c + 1])
                s1s = slice(f1_src_cuts[c], f1_src_cuts[c + 1])
                s1d = slice(f1_src_cuts[c] + MD * WC,
                            f1_src_cuts[c + 1] + MD * WC)
                # all input chunks ride the SP queue in exact consumption
                # order: a chunk's completion sem fires only once all 16 DMA
                # engines finish it, so cross-queue interleaving would delay
                # early chunks behind later traffic (measured 2.6 us).
                nc.sync.dma_start(out=f2_dst[:, :, s2], in_=f2_src[:, :, s2])
                nc.sync.dma_start(out=f1_dst[:, :, s1d], in_=f1_src[:, :, s1s])

            band = [persist.tile([128, 2 * NWIN], bf16, tag=f"band{p}",
                                 name=f"band{p}")
                    for p in range(NPAIR)]

            for k in range(NQ):
                kr, kc = divmod(k, 2)
                ps = psum_pool.tile([128, NWIN], f32, tag="ps")
                for h in range(2):
                    win = (f1_t[:, h * HP * WC:(h + 1) * HP * WC]
                           .rearrange("p (r c) -> p r c", c=WC)
                           [:, RQ * kr:RQ * kr + RB, CQ * kc:CQ * kc + CB])
                    nc.tensor.matmul(
                        ps[:],
                        f2_t[:, h * H * CSH + 128 * k:
                             h * H * CSH + 128 * (k + 1)],
                        win,
                        start=(h == 0), stop=(h == 1),
                    )
                dst = band[kr][:, kc * NWIN:(kc + 1) * NWIN]
                if k % 2:
                    nc.scalar.copy(out=dst, in_=ps[:])
                else:
                    nc.vector.tensor_copy(out=dst, in_=ps[:])

                if kc == 1:
                    nc.sync.dma_start(
                        out=outd[2 * kr:2 * (kr + 1)].transpose([1, 0, 2]),
                        in_=band[kr][:, :])
                if k in (1,):
                    for _ in range(3):
                        wp = psum_pool.tile([128, NWIN], f32, tag="ps",
                                            name="wp2")
                        nc.tensor.matmul(wp[:], warm_t[:, 0:128], warm_t[:],
                                         start=True, stop=True)

    nc.compile()
    return nc


def _shard_inputs(feat1, feat2):
    """Per-core input dicts. Core i = (batch i//2, column-half i%2)."""
    import ml_dtypes
    bf = ml_dtypes.bfloat16
    in_maps = []
    for i in range(N_CORES):
        b, ch = divmod(i, 2)
        clo = ch * CSH
        # f2 in quad-partition order: (ch, kr, kc, rg, cj)
        f2s = (feat2[b, :, :, clo:clo + CSH]
               .reshape(C, H // RQ, RQ, CSH // CQ, CQ)
               .transpose(0, 1, 3, 2, 4)
               .reshape(C, H * CSH)).astype(bf)
        f2s = np.ascontiguousarray(f2s)
        # f1: columns [clo-4, clo+36) with zeros outside the image; the
        # 4 pad rows top/bottom are memset on-device
        f1p = np.zeros((C, H, WC), np.float32)
        lo, hi = max(0, clo - MD), min(W, clo + CSH + MD)
        f1p[:, :, lo - (clo - MD):hi - (clo - MD)] = feat1[b, :, :, lo:hi]
        in_maps.append({"f2": f2s, "f1": f1p.reshape(C, H * WC).astype(bf)})
    return in_maps


def _unshard(results):
    """Scatter the per-core bf16 bands into the dense (B,H,W,H,W) output."""
    P = 2 * MD + 1
    V = np.empty((B, H, W, P, P), np.float32)
    for i in range(N_CORES):
        b, ch = divmod(i, 2)
        a = np.asarray(results[i]["out"]).astype(np.float32)
        a = a.reshape(H // RQ, CSH // CQ, RQ, CQ, RB, CB)
        s = a.strides
        # Vc[kr, kc, rg, cj, di, dj] = a[kr, kc, rg, cj, rg + di, cj + dj]
        Vc = np.lib.stride_tricks.as_strided(
            a, shape=(H // RQ, CSH // CQ, RQ, CQ, P, P),
            strides=(s[0], s[1], s[2] + s[4], s[3] + s[5], s[4], s[5]))
        V[b, :, ch * CSH:(ch + 1) * CSH] = (
            Vc.transpose(0, 2, 1, 3, 4, 5).reshape(H, CSH, P, P))

    out5 = np.zeros((B, H, W, H, W), np.float32)
    so = out5.strides
    for di in range(P):
        ddi = di - MD
        rlo, rhi = max(0, -ddi), min(H, H - ddi)
        for dj in range(P):
            ddj = dj - MD
            clo2, chi2 = max(0, -ddj), min(W, W - ddj)
            src = V[:, rlo:rhi, clo2:chi2, di, dj]
            base = out5[:, rlo:, clo2:, rlo + ddi:, clo2 + ddj:]
            tgt = np.lib.stride_tricks.as_strided(
                base, shape=(B, rhi - rlo, chi2 - clo2),
                strides=(so[0], so[1] + so[3], so[2] + so[4]))
            tgt[...] = src
    return out5.reshape(B, H * W, H, W)


def run(feat1, feat2, trace=False, trace_cores=None):
    """Returns (full output (B, H*W, H, W) float32, exec_time_ns or None)."""
    global _COMPILED
    from concourse.bass_utils import run_bass_kernel_spmd

    feat1 = np.asarray(feat1, dtype=np.float32)
    feat2 = np.asarray(feat2, dtype=np.float32)
    assert feat1.shape == (B, C, H, W) and feat2.shape == (B, C, H, W)

    if _COMPILED is None:
        _COMPILED = _build_program()
    nc = _COMPILED

    in_maps = _shard_inputs(feat1, feat2)
    res = run_bass_kernel_spmd(
        nc, in_maps, core_ids=list(range(N_CORES)),
        trace=trace, trace_cores=trace_cores,
    )
    return _unshard(res.results), res.exec_time_ns


def kernel(feat1, feat2):
    out, _ = run(feat1, feat2, trace=False)
    return out
